# revision 38
# baseline (speedup 1.0000x reference)
"""ResNet BasicBlock (conv3x3-BN-ReLU-conv3x3-BN-+res-ReLU) on 8 trn2 NeuronCores.

Data-parallel over the batch (4 images per core). BatchNorm uses global batch
statistics, reduced across cores with a small AllGather.

Per-core layout: channels on partitions; partitions 0-63 hold images {0,1} of
the core's shard, partitions 64-127 images {2,3}. Each 3x3 conv is 9 shifted
matmuls accumulating in PSUM (bf16 operands, fp32 PSUM accumulation). The
stationary weight is a 128x128 block-diagonal matrix (the 64x64 conv weight
duplicated on the diagonal), so a single matmul per tap convolves both image
halves and writes all 128 PSUM partitions at once.

Feature planes are bf16, 60 columns wide with TWO zero border columns on each
side so every elementwise slice of the 56-column interior starts 4B-aligned
(bf16 DVE/Act packing needs that). Horizontal taps read src cols
[tx+1, tx+57); vertical taps use valid-row ranges instead of row padding.
The conv2 input's border columns are preset to -1e9 so the full-row
relu(scale*z+shift) pass writes exact zeros into them - no re-zeroing pass.

Inputs are pre-packed host-side into the exact SBUF layouts (x pre-padded,
pre-swizzled bf16; weights as the full block-diagonal bf16 matrix) so every
DMA is a large contiguous-run transfer; x streams in row-chunks prioritized
in conv group order so the first matmul issues ~2us in. Output is written
bf16 and widened host-side.
"""
import numpy as np
import ml_dtypes
from contextlib import ExitStack

import concourse.bass as bass
import concourse.bacc as bacc
import concourse.mybir as mybir
import concourse.tile as tile
from concourse.bass_utils import run_bass_kernel_spmd

N_CORES = 8
B, C, H, W = 32, 64, 56, 56
BL = B // N_CORES           # images per core
P = 64                      # conv output channels
PW = W + 4                  # column-padded plane width (2 pads each side)
EPS = 1e-5
RB = 4                      # output rows per chunk
NCHUNK = H // RB            # 14
NTOT = float(B * H * W)     # BN normalization count

f32 = mybir.dt.float32
bf16 = mybir.dt.bfloat16
AF = mybir.ActivationFunctionType
ALU = mybir.AluOpType
AX = mybir.AxisListType

# center tap first: it is full-coverage for every chunk, so its start=True
# clears the whole PSUM bank before the partial edge taps accumulate.
TAPS = [(1, 1), (0, 0), (0, 1), (0, 2), (1, 0), (1, 2), (2, 0), (2, 1), (2, 2)]

GROUPS = (2, 4, 4, 3, 1)                  # conv chunks per psum group; small
                                          # tail groups so the eviction +
                                          # bn_stats backlog after the last
                                          # matmul stays short
XCHUNKS = ((0, 5), (5, 9), (9, 25), (25, 56))   # x row-chunk DMA order
RSLABS = ((0, 5), (5, 9), (9, 25), (25, 41), (41, 56))  # conv2-group order


def build(n_cores=N_CORES, use_collective=True):
    nc = bacc.Bacc(
        "TRN2", target_bir_lowering=False, debug=False,
        enable_asserts=False, num_devices=n_cores,
    )
    xs_d = nc.dram_tensor("xs", [128, 2, H, PW], bf16, kind="ExternalInput")
    w1_d = nc.dram_tensor("w1p", [128, 9, 128], bf16, kind="ExternalInput")
    w2_d = nc.dram_tensor("w2p", [128, 9, 128], bf16, kind="ExternalInput")
    bn_d = nc.dram_tensor("bn", [128, 4], f32, kind="ExternalInput")
    out_d = nc.dram_tensor("out", [BL, C, H, W], bf16, kind="ExternalOutput")

    with tile.TileContext(nc) as tc:
        with ExitStack() as ctx:
            main = ctx.enter_context(tc.tile_pool(name="main", bufs=1))
            psum = ctx.enter_context(tc.tile_pool(name="psum", bufs=1, space="PSUM"))
            smal = ctx.enter_context(tc.tile_pool(name="smal", bufs=1))
            dram = ctx.enter_context(tc.tile_pool(name="dram", bufs=1, space="DRAM"))

            x_sb = main.tile([128, 2, H, PW], bf16)
            z_sb = main.tile([128, 2, H, PW], bf16)
            y2 = main.tile([128, 2, H, W], bf16)
            fin = main.tile([128, 2, H, W], bf16)
            w1s = main.tile([128, 9, 128], bf16)
            w2s = main.tile([128, 9, 128], bf16)
            gb = main.tile([128, 4], f32)
            sp1 = main.tile([128, NCHUNK, 6], f32)
            sp2 = main.tile([128, NCHUNK, 6], f32)

            # weights/bn on the ACT HWDGE ring, x row-chunks on the SP ring,
            # ordered so conv1's first group unblocks as early as possible.
            # w1 is issued before the act-table preload so it isn't queued
            # behind the ~1.3us table load on the ACT sequencer.
            nc.scalar.dma_start(w1s[:], w1_d[:])
            for (r0, r1) in XCHUNKS:
                nc.sync.dma_start(x_sb[:, :, r0:r1, :], xs_d[:, :, r0:r1, :])

            # ACT table preload (sqrt set also carries relu/copy) so the
            # table DMA overlaps the input loads instead of landing on the BN
            # critical path.
            dumm = smal.tile([128, 1], f32, name="dumm")
            nc.vector.memset(dumm[:], 1.0)
            dum2 = smal.tile([128, 1], f32, name="dum2")
            nc.scalar.activation(dum2[:], dumm[:], AF.Sqrt)
            nc.scalar.activation(dum2[:], dumm[:], AF.Relu)

            nc.scalar.dma_start(gb[:], bn_d[:])
            nc.scalar.dma_start(w2s[:], w2_d[:])

            # conv2-input border columns: -1e9, so relu(scale*z+shift) writes
            # exact zeros there (scale > 0); x_sb comes pre-padded from host.
            for cols in (0, 2), (PW - 2, PW):
                nc.vector.memset(z_sb[:, :, :, cols[0]:cols[1]], -1e9)

            def conv(src, wsb, evict):
                cg0 = 0
                for cn in GROUPS:
                    pss = [psum.tile([128, 2, RB, W], f32, name="ps", tag="ps",
                                     bufs=8) for _ in range(cn)]
                    for k, (ty, tx) in enumerate(TAPS):
                        dy = ty - 1
                        st = k == 0
                        sp = k == len(TAPS) - 1
                        for ci in range(cn):
                            r0 = RB * (cg0 + ci)
                            y0 = max(r0, -dy)
                            y1 = min(r0 + RB, H - dy)
                            il, ih = y0 - r0, y1 - r0
                            nc.tensor.matmul(
                                pss[ci][:, :, il:ih, :],
                                wsb[:, 3 * ty + tx, :],
                                src[:, :, y0 + dy:y1 + dy, tx + 1:tx + 1 + W],
                                start=st, stop=sp)
                    for ci in range(cn):
                        evict(cg0 + ci, pss[ci])
                    cg0 += cn

            def evict1(c, ps):
                r0 = RB * c
                nc.scalar.activation(
                    z_sb[:, :, r0:r0 + RB, 2:2 + W], ps[:], AF.Copy)
                nc.vector.bn_stats(sp1[:, c, :],
                                   ps[:].rearrange("p a b c -> p (a b c)"))

            def evict2(c, ps):
                r0 = RB * c
                nc.scalar.activation(y2[:, :, r0:r0 + RB, :], ps[:], AF.Copy)
                nc.vector.bn_stats(sp2[:, c, :],
                                   ps[:].rearrange("p a b c -> p (a b c)"))

            def bn_sync(sparts, gcol, idx):
                # fold the per-chunk bn_stats triples into local (sum, sumsq)
                t = sparts[:].rearrange("p c (g v) -> p (c g) v", v=3)
                nt = NCHUNK * 2
                cm = smal.tile([128, nt], f32, name=f"cm{idx}")
                nc.vector.tensor_mul(cm[:], t[:, :, 0], t[:, :, 1])
                qq = smal.tile([128, nt], f32, name=f"qq{idx}")
                nc.vector.tensor_mul(qq[:], t[:, :, 1], t[:, :, 1])
                nc.vector.tensor_mul(qq[:], qq[:], t[:, :, 0])
                nc.vector.tensor_add(qq[:], qq[:], t[:, :, 2])
                loc = smal.tile([128, 2], f32, name=f"loc{idx}")
                nc.vector.tensor_reduce(loc[:, 0:1], cm[:], axis=AX.X, op=ALU.add)
                nc.vector.tensor_reduce(loc[:, 1:2], qq[:], axis=AX.X, op=ALU.add)

                cc_in = dram.tile([128, 2], f32, name=f"ccin{idx}")
                cc_out = dram.tile([N_CORES * 128, 2], f32, name=f"ccout{idx}",
                                   addr_space="Shared")
                nc.sync.dma_start(cc_in[:], loc[:])
                if use_collective:
                    nc.gpsimd.collective_compute(
                        "AllGather", ALU.bypass,
                        replica_groups=[list(range(N_CORES))],
                        ins=[cc_in[:].opt()], outs=[cc_out[:].opt()],
                    )
                else:
                    # timing-only A/B stub: local DRAM round-trip in place of
                    # the AllGather (numerics intentionally wrong)
                    nc.sync.dma_start(cc_out[0:128], cc_in[:])
                gath = smal.tile([128, 16, 2], f32, name=f"gath{idx}")
                src = cc_out[:].rearrange("(j p) v -> p j v", p=64)
                nc.sync.dma_start(gath[0:64], src)
                nc.scalar.dma_start(gath[64:128], src)
                gs = smal.tile([128, 2], f32, name=f"gs{idx}")
                nc.vector.tensor_reduce(
                    gs[:], gath[:].rearrange("p j v -> p v j"),
                    axis=AX.X, op=ALU.add)

                # mean/var -> scale/shift (per partition, tiny ops)
                mv = smal.tile([128, 2], f32, name=f"mv{idx}")
                nc.vector.tensor_scalar_mul(mv[:], gs[:], 1.0 / NTOT)
                m2 = smal.tile([128, 1], f32, name=f"m2{idx}")
                nc.vector.tensor_mul(m2[:], mv[:, 0:1], mv[:, 0:1])
                var = smal.tile([128, 1], f32, name=f"var{idx}")
                nc.vector.scalar_tensor_tensor(
                    var[:], mv[:, 1:2], EPS, m2[:], op0=ALU.add, op1=ALU.subtract)
                inv = smal.tile([128, 1], f32, name=f"inv{idx}")
                nc.vector.reciprocal(inv[:], var[:])
                istd = smal.tile([128, 1], f32, name=f"istd{idx}")
                nc.scalar.activation(istd[:], inv[:], AF.Sqrt)
                sc = smal.tile([128, 1], f32, name=f"sc{idx}")
                nc.vector.tensor_mul(sc[:], gb[:, gcol:gcol + 1], istd[:])
                sh = smal.tile([128, 1], f32, name=f"sh{idx}")
                nc.vector.tensor_mul(sh[:], mv[:, 0:1], sc[:])
                nc.vector.tensor_sub(sh[:], gb[:, gcol + 1:gcol + 2], sh[:])
                return sc, sh, gath

            # ---- conv1 -> BN1 stats sync -> relu(bn1) in place ----
            conv(x_sb, w1s, evict1)
            sc1, sh1, _ = bn_sync(sp1, 0, 1)
            for (r0, r1) in RSLABS:
                # full rows incl. border cols: -1e9 pads relu to exact 0,
                # and the aligned even-width slab keeps bf16 Act packing.
                zint = z_sb[:, :, r0:r1, :]
                nc.scalar.activation(zint, zint, AF.Relu,
                                     bias=sh1[:], scale=sc1[:])

            # ---- conv2 -> BN2 stats sync -> fused residual tail ----
            conv(z_sb, w2s, evict2)
            sc2, sh2, _ = bn_sync(sp2, 2, 2)
            TG = 14
            for rb in range(0, H, TG):
                for j in range(2):
                    y2g = y2[:, j, rb:rb + TG, :]
                    fing = fin[:, j, rb:rb + TG, :]
                    xg = x_sb[:, j, rb:rb + TG, 2:2 + W]
                    nc.vector.scalar_tensor_tensor(
                        fing, y2g, sc2[:], xg, op0=ALU.mult, op1=ALU.add)
                    # balance the relu passes: Act handles 5 of 8, DVE 3
                    if j == 0 or rb >= 2 * TG:
                        nc.scalar.activation(y2g, fing, AF.Relu, bias=sh2[:])
                    else:
                        nc.vector.tensor_scalar(y2g, fing, sh2[:], 0.0,
                                                op0=ALU.add, op1=ALU.max)
                # one DMA per image pair: out[2hh:2hh+2] viewed [c, j, h, w]
                for hh in range(2):
                    dst = out_d[2 * hh:2 * hh + 2, :, rb:rb + TG, :].rearrange(
                        "j c h w -> c j h w")
                    nc.sync.dma_start(
                        dst, y2[64 * hh:64 * hh + 64, :, rb:rb + TG, :])

    nc.compile()
    return nc


_CACHE = {}


def _get_nc():
    if "nc" not in _CACHE:
        _CACHE["nc"] = build()
    return _CACHE["nc"]


def make_in_maps(x, w1, b1, g1, be1, w2, b2, g2, be2):
    """Shard + pre-pack host-side into the exact SBUF layouts. Conv biases
    b1/b2 cancel exactly through the batch-norms (bn(x + c) == bn(x)), so
    they are dropped."""
    x = np.asarray(x, np.float32)

    # x: [32,C,H,W] -> per core [128, 2, H, 60] bf16, partition p = hh*64+c,
    # image b_local = 2*hh + j, two zero border columns each side.
    xp = np.zeros((N_CORES, 2, C, 2, H, PW), ml_dtypes.bfloat16)
    xp[:, :, :, :, :, 2:2 + W] = x.reshape(
        N_CORES, 2, 2, C, H, W).transpose(0, 1, 3, 2, 4, 5)
    xp = np.ascontiguousarray(xp.reshape(N_CORES, 128, 2, H, PW))

    def packw(w):
        wt = np.asarray(w, np.float32).transpose(2, 3, 1, 0)  # [kh,kw,I,O]
        wt = wt.reshape(9, C, P).transpose(1, 0, 2)           # [c, t, o]
        full = np.zeros((128, 9, 128), ml_dtypes.bfloat16)
        full[0:64, :, 0:64] = wt
        full[64:128, :, 64:128] = wt
        return full

    def packbn(g, be):
        g = np.asarray(g, np.float32)
        be = np.asarray(be, np.float32)
        return np.stack([np.concatenate([g, g]),
                         np.concatenate([be, be])], axis=1)

    w1p, w2p = packw(w1), packw(w2)
    bn = np.ascontiguousarray(
        np.concatenate([packbn(g1, be1), packbn(g2, be2)], axis=1))
    return [
        {"xs": xp[r], "w1p": w1p, "w2p": w2p, "bn": bn}
        for r in range(N_CORES)
    ]


def kernel(x, w1, b1, g1, be1, w2, b2, g2, be2):
    nc = _get_nc()
    in_maps = make_in_maps(x, w1, b1, g1, be1, w2, b2, g2, be2)
    res = run_bass_kernel_spmd(nc, in_maps, core_ids=list(range(N_CORES)))
    return np.concatenate(
        [res.results[r]["out"].astype(np.float32) for r in range(N_CORES)],
        axis=0)


if __name__ == "__main__":
    rng = np.random.default_rng(0)
    ins = {
        "x": rng.standard_normal((B, C, H, W)).astype(np.float32),
        "w1": rng.standard_normal((P, C, 3, 3)).astype(np.float32) * 0.04,
        "b1": rng.standard_normal((P,)).astype(np.float32) * 0.04,
        "g1": np.ones((P,), np.float32), "be1": np.zeros((P,), np.float32),
        "w2": rng.standard_normal((P, P, 3, 3)).astype(np.float32) * 0.04,
        "b2": rng.standard_normal((P,)).astype(np.float32) * 0.04,
        "g2": np.ones((P,), np.float32), "be2": np.zeros((P,), np.float32),
    }
    out = kernel(**ins)
    print("out", out.shape, out.dtype, float(np.abs(out).mean()))
